# revision 14
# baseline (speedup 1.0000x reference)
"""Causal multi-head attention block (QKV proj + flash-style attention + out proj)
for Trainium2, sharded over 8 NeuronCores as (batch, head-group):
core c -> batch b = c//2, heads hg*4..hg*4+4 with hg = c%2.

Row-split precision hybrid per 512-wide q-block:
  qb0 (rows 0:512, concentrated softmax): bf16 everywhere (error-sensitive).
  qb1-3 (rows 512:2048): fp8(e4m3) DoubleRow matmuls for S, P@V and the
    row-sum L; exp is computed as exp(S*SCALE - SHIFT) so P <= ~60 stays
    inside TRN e4m3 range (+-240); the shift cancels in O/l exactly.
  Row sums l are computed with an all-ones [128,k] stationary so the PE
  broadcasts l to all 128 partitions (no gpsimd partition-broadcast needed).
  Out projection runs transposed (w_proj stationary, O^T moving) for
  stationary reuse; host transposes the (C, N) result back.
"""

import numpy as np
import ml_dtypes

import concourse.bass as bass
import concourse.tile as tile
from concourse import bacc, mybir
from concourse.bass_utils import run_bass_kernel_spmd

B, N, C, H = 4, 2048, 256, 8
SCALE = C ** -0.5
SHIFT = 2.0
BF16 = ml_dtypes.bfloat16
FP32 = mybir.dt.float32
BF = mybir.dt.bfloat16
F8 = mybir.dt.float8e4
HPC = 4  # heads per core
DR = mybir.MatmulPerfMode.DoubleRow


def _emit(tc, nq, aps):
    nc = tc.nc
    nt = nq // 128

    xt_d, wqkv_d, wproj_d, bias_d, mask_d, ztri_d, out_d = aps
    xt_r = xt_d.rearrange("(c p) n -> p c n", p=128)
    wqkv_r = wqkv_d.rearrange("(c p) m -> p c m", p=128)
    wproj_r = wproj_d.rearrange("(t p) f -> p t f", p=128)
    out_r = out_d.rearrange("(t p) n -> p t n", p=128)

    singles = tc._es.enter_context(tc.tile_pool(name="singles", bufs=1))
    pool_head = tc._es.enter_context(tc.tile_pool(name="headp", bufs=2))
    pool_p16 = tc._es.enter_context(tc.tile_pool(name="p16p", bufs=3))
    pool_p8 = tc._es.enter_context(tc.tile_pool(name="p8p", bufs=3))
    pool_rb = tc._es.enter_context(tc.tile_pool(name="rbp", bufs=2))
    pool_osb = tc._es.enter_context(tc.tile_pool(name="osbp", bufs=4))
    # PSUM: sp 2x2 banks + ot0/ot1 2 + l 1 + qp 1 = 8 banks
    pool_sp = tc._es.enter_context(tc.tile_pool(name="psumsp", bufs=2, space="PSUM"))
    pool_ot = tc._es.enter_context(tc.tile_pool(name="psumot", bufs=1, space="PSUM"))
    pool_l = tc._es.enter_context(tc.tile_pool(name="psuml", bufs=1, space="PSUM"))
    pool_qp = tc._es.enter_context(tc.tile_pool(name="psumqp", bufs=1, space="PSUM"))

    # --- SBUF constants / inputs ---
    xt_sb = singles.tile([128, 2, nq], BF)
    wqkv_sb = singles.tile([128, 2, 3 * HPC * C], BF)
    xt8_sb = singles.tile([128, 2, nq], F8)
    wqkv8_sb = singles.tile([128, 2, 2 * HPC * C], F8)  # q,k cols only
    wproj_sb = singles.tile([128, 2 * HPC, C], BF)
    bias_sb = singles.tile([128, 2], FP32)
    mask_sb = singles.tile([128, 128], BF)
    ztri_sb = singles.tile([128, 256], BF)
    tri8_sb = singles.tile([128, 128], F8)
    ztri8_sb = singles.tile([128, 256], F8)
    ones16_sb = singles.tile([128, 128], BF)
    ones8_sb = singles.tile([128, 2, 128], F8)
    shiftb_sb = singles.tile([128, 1], FP32)
    ot_sb = singles.tile([128, 2 * HPC, nq], BF)

    # input DMAs split across both HW-DGE rings; the small mask DMA goes
    # first so the PE warm-up (which reads it) can start ~3us in
    nc.sync.dma_start(mask_sb[:], mask_d[:])
    for ib in range(4):
        nc.sync.dma_start(xt_sb[:, :, ib * 512:(ib + 1) * 512],
                          xt_r[:, :, ib * 512:(ib + 1) * 512])
    for hw in range(HPC):
        c0 = hw * 3 * C
        nc.scalar.dma_start(wqkv_sb[:, :, c0:c0 + 3 * C], wqkv_r[:, :, c0:c0 + 3 * C])
    nc.sync.dma_start(ztri_sb[:], ztri_d[:])
    nc.scalar.dma_start(wproj_sb[:], wproj_r)
    nc.scalar.dma_start(bias_sb[:], bias_d[:])
    nc.vector.tensor_copy(tri8_sb[:], mask_sb[:])
    nc.vector.tensor_copy(ztri8_sb[:], ztri_sb[:])
    nc.gpsimd.memset(ones16_sb[:], 1.0)
    nc.gpsimd.memset(ones8_sb[:], 1.0)
    nc.gpsimd.memset(shiftb_sb[:], -SHIFT)

    # warm up the PE HAM clock gate while the big input DMAs land; gated
    # only on the tiny mask DMA so the matmuls start ~3us into the kernel
    warm_ps = pool_qp.tile([128, 512], FP32, tag="qp", name="warm_ps")
    for wi in range(12):
        nc.tensor.matmul(warm_ps[:, :128], mask_sb[:], mask_sb[:],
                         start=(wi == 0), stop=(wi == 11))

    def alloc_head_tiles():
        qt16 = pool_head.tile([128, 2, 512], BF, tag="qt16", name="qt16")
        kt16 = pool_head.tile([128, 2, 512], BF, tag="kt16", name="kt16")
        qt8 = pool_head.tile([128, 2, nq], F8, tag="qt8", name="qt8")
        kt8 = pool_head.tile([128, 2, nq], F8, tag="kt8", name="kt8")
        v16 = pool_head.tile([128, 4, C], BF, tag="v16", name="v16")
        v8 = pool_head.tile([128, nt, C], F8, tag="v8", name="v8")
        return qt16, kt16, qt8, kt8, v16, v8

    par = {"i": 0}

    def qkv_blocks(hp, tiles, psum_alloc, act_ok):
        """Per-(128x512)-block closures for head hp's QKV projection.
        psum_alloc() -> a [128, 512] fp32 PSUM tile.  act_ok: may use the
        Scalar engine for casts (only before attention starts; during
        attention ACT must stay exp-only or drip casts stall the PE)."""
        qt16, kt16, qt8, kt8, v16, v8 = tiles
        blocks = []

        def cast(dst, src):
            par["i"] += 1
            if act_ok and par["i"] % 2 == 0:
                nc.scalar.copy(dst, src)
            else:
                nc.vector.tensor_copy(dst, src)

        def qk_block(j, ct, ib):
            def go():
                ps = psum_alloc()
                tgt8 = qt8 if j == 0 else kt8
                if ib == 0:
                    col0 = (hp * 3 + j) * C + ct * 128
                    for ci in range(2):
                        nc.tensor.matmul(
                            ps[:], wqkv_sb[:, ci, col0:col0 + 128],
                            xt_sb[:, ci, ib * 512:(ib + 1) * 512],
                            start=(ci == 0), stop=(ci == 1),
                        )
                    tgt16 = qt16 if j == 0 else kt16
                    cast(tgt16[:, ct, :], ps[:])
                    if j == 1:
                        cast(tgt8[:, ct, 0:512], ps[:])
                else:
                    col8 = (hp * 2 + j) * C + ct * 128
                    nc.tensor.matmul(
                        ps[:], wqkv8_sb[:, :, col8:col8 + 128],
                        xt8_sb[:, :, ib * 512:(ib + 1) * 512],
                        start=True, stop=True, perf_mode=DR,
                    )
                    cast(tgt8[:, ct, ib * 512:(ib + 1) * 512], ps[:])
            return go

        def v_block(it):
            def go():
                ps = psum_alloc()
                vcol = (hp * 3 + 2) * C
                for ci in range(2):
                    nc.tensor.matmul(
                        ps[:, :C], xt_sb[:, ci, it * 128:(it + 1) * 128],
                        wqkv_sb[:, ci, vcol:vcol + C],
                        start=(ci == 0), stop=(ci == 1),
                    )
                cast(v8[:, it, :], ps[:, :C])
                if it < 4:
                    cast(v16[:, it, :], ps[:, :C])
            return go

        # order: qb0-critical first (q/k ib0, v it0-3), then fp8 operands
        for j in (0, 1):
            for ct in range(2):
                blocks.append((0, qk_block(j, ct, 0)))
        for it in range(4):
            blocks.append((0, v_block(it)))
        for j in (1, 0):
            for ct in range(2):
                for ib in range(1, 4):
                    blocks.append((0, qk_block(j, ct, ib)))
        for it in range(4, nt):
            blocks.append((0, v_block(it)))
        return blocks

    otl_rot = {"i": 0}
    otl_tags = [(pool_ot, "ot0"), (pool_ot, "ot1"), (pool_l, "l")]
    SLOTS_PER_HEAD = 22

    def head_slots(hp):
        slots = []
        for kt in range(4):  # qb0, bf16, one kt per slot
            q_off = kt * 128
            slots.append(dict(hp=hp, kind=16, qb=0, kt=kt, q_off=q_off,
                              nqf=512 - q_off, first=(kt == 0), last=(kt == 3),
                              diag=True))
        for qb in range(1, 4):  # fp8 pairs
            npair = 2 * qb + 2
            for j in range(npair):
                q_off = 256 if j == npair - 1 else 0
                slots.append(dict(hp=hp, kind=8, qb=qb, j=j, q_off=q_off,
                                  nqf=512 - q_off, first=(j == 0),
                                  last=(j == npair - 1), diag=(j >= 2 * qb)))
        return slots

    state = {}

    def otl_tiles():
        """Allocate (ot0, ot1, lp) with per-qb tag rotation so each new
        allocation lands on the earliest-freed PSUM bank."""
        r = otl_rot["i"]
        otl_rot["i"] += 1
        tags = [otl_tags[(r + k) % 3] for k in range(3)]
        # emission order per first slot: ot1 MM, ot0 MM, L MM
        ot1 = tags[0][0].tile([128, 512], FP32, tag=tags[0][1], name="ot1")
        ot0 = tags[1][0].tile([128, 512], FP32, tag=tags[1][1], name="ot0")
        lp = tags[2][0].tile([128, 512], FP32, tag=tags[2][1], name="lp")
        return ot0, ot1, lp

    def emit_S(s, T):
        qt16, kt16, qt8, kt8, v16, v8 = T
        sp = pool_sp.tile([128, 2, 512], FP32, tag="sp", name="sp")
        nqf = s["nqf"]
        if s["kind"] == 16:
            q0 = s["q_off"]
            for ci in range(2):
                nc.tensor.matmul(
                    sp[:, 0, :nqf], kt16[:, ci, s["kt"] * 128:(s["kt"] + 1) * 128],
                    qt16[:, ci, q0:q0 + nqf], start=(ci == 0), stop=(ci == 1),
                )
            p = pool_p16.tile([128, 512], BF, tag="p16", name="p16")
            nc.scalar.activation(p[:, :nqf], sp[:, 0, :nqf],
                                 mybir.ActivationFunctionType.Exp, scale=SCALE)
            nc.vector.tensor_tensor(p[:, :128], p[:, :128], mask_sb[:],
                                    mybir.AluOpType.mult)
        else:
            q0 = s["qb"] * 512 + s["q_off"]
            p = pool_p8.tile([128, 2, 512], F8, tag="p8", name="p8")
            for i in range(2):
                kt = 2 * s["j"] + i
                nc.tensor.matmul(
                    sp[:, i, :nqf], kt8[:, :, kt * 128:(kt + 1) * 128],
                    qt8[:, :, q0:q0 + nqf], start=True, stop=True, perf_mode=DR,
                )
            nc.scalar.activation(p[:, :, :nqf], sp[:, :, :nqf],
                                 mybir.ActivationFunctionType.Exp,
                                 scale=SCALE, bias=shiftb_sb[:])
            if s["diag"]:
                nc.vector.tensor_tensor(p[:, 0, 0:128], p[:, 0, 0:128],
                                        tri8_sb[:], mybir.AluOpType.mult)
                nc.vector.tensor_tensor(p[:, 1, 0:256], p[:, 1, 0:256],
                                        ztri8_sb[:], mybir.AluOpType.mult)
        return p

    def emit_PVL(s, p, T):
        qt16, kt16, qt8, kt8, v16, v8 = T
        hp, qb, q_off, nqf = s["hp"], s["qb"], s["q_off"], s["nqf"]
        first, last = s["first"], s["last"]
        if first:
            state[(hp, qb)] = (*otl_tiles(), {})
        ot0, ot1, lp, aux = state[(hp, qb)]
        # groups close ("stop") once cols [0:256) are final so the half
        # normalize may read them; later matmuls keep accumulating the
        # [256:512) range (stop is sim bookkeeping only, no HW effect)
        if s["kind"] == 16:
            kt = s["kt"]
            stop_f = (kt == 1) or last
            skip = kt >= 2
            nc.tensor.matmul(ot1[:, q_off:], v16[:, kt, 0:128], p[:, :nqf],
                             start=first, stop=stop_f, skip_group_check=skip)
            nc.tensor.matmul(ot0[:, q_off:], v16[:, kt, 128:256], p[:, :nqf],
                             start=first, stop=stop_f, skip_group_check=skip)
            nc.tensor.matmul(lp[:, q_off:], ones16_sb[:], p[:, :nqf],
                             start=first, stop=stop_f, skip_group_check=skip)
        else:
            j2 = 2 * s["j"]
            stop_f = s["diag"]
            skip = last
            nc.tensor.matmul(ot1[:, q_off:], v8[:, j2:j2 + 2, 0:128],
                             p[:, :, :nqf], start=first, stop=stop_f,
                             skip_group_check=skip, perf_mode=DR)
            nc.tensor.matmul(ot0[:, q_off:], v8[:, j2:j2 + 2, 128:256],
                             p[:, :, :nqf], start=first, stop=stop_f,
                             skip_group_check=skip, perf_mode=DR)
            nc.tensor.matmul(lp[:, q_off:], ones8_sb[:],
                             p[:, :, :nqf], start=first, stop=stop_f,
                             skip_group_check=skip, perf_mode=DR)
        # normalize in halves: cols [0:256) are final one slot before the
        # last (the final pair/step only writes cols >= 256), which spreads
        # the DVE work and frees the ot/l banks earlier
        def half_norm(lo, hi):
            if aux.get("rb") is None:
                aux["rb"] = pool_rb.tile([128, 512], FP32, tag="rb", name="rb")
            rb = aux["rb"]
            nc.vector.reciprocal_approx_fast(rb[:, lo:hi], lp[:, lo:hi])
            for ct, otp in ((0, ot1), (1, ot0)):
                nc.vector.tensor_tensor(
                    ot_sb[:, hp * 2 + ct, qb * 512 + lo:qb * 512 + hi],
                    otp[:, lo:hi], rb[:, lo:hi], mybir.AluOpType.mult,
                )

        if (s["kind"] == 16 and s["kt"] == 1) or (s["kind"] == 8 and s["diag"]
                                                  and not last):
            half_norm(0, 256)
        if last:
            half_norm(256, 512)

    # --- head 0 QKV with a deep temporary psum ring (attention not started) ---
    h0_ring = {"i": 0}
    h0_tags = [(pool_qp, "qp"), (pool_ot, "ot0"), (pool_ot, "ot1"), (pool_l, "l")]

    def h0_psum():
        pool, tag = h0_tags[h0_ring["i"] % 4]
        h0_ring["i"] += 1
        return pool.tile([128, 512], FP32, tag=tag, name="h0qkv")

    def drip_psum():
        return pool_qp.tile([128, 512], FP32, tag="qp", name="qkvps")

    def proj_units():
        """Output projection out^T[f, n] = sum_t W[t]^T O^T[t] + b, one unit
        per (f-chunk, n-chunk), dripped into head 3's attention as soon as
        head 3's q-block n-chunk has been normalized."""
        units = []

        def unit(f, nch):
            def go():
                ps = pool_qp.tile([128, 512], FP32, tag="qp", name="ps_prj")
                for t in range(2 * HPC):
                    nc.tensor.matmul(
                        ps[:], wproj_sb[:, t, f * 128:(f + 1) * 128],
                        ot_sb[:, t, nch * 512:(nch + 1) * 512],
                        start=(t == 0), stop=(t == 2 * HPC - 1),
                    )
                osb = pool_osb.tile([128, 512], FP32, tag="osb", name="osb")
                nc.vector.tensor_scalar_add(osb[:], ps[:], bias_sb[:, f:f + 1])
                nc.sync.dma_start(out_r[:, f, nch * 512:(nch + 1) * 512], osb[:])
            return go

        mins = {0: 6, 1: 10, 2: 16, 3: 22}
        for nch in range(4):
            for f in range(2):
                units.append((mins[nch], unit(f, nch)))
        return units

    # --- global slot stream: the lag-2 S->PVL pipeline runs across all four
    # heads so there is no per-head drain; head h+1's QKV (and the output
    # projection during head 3) drip into the stream paced by min-slot ---
    all_tiles = [alloc_head_tiles() for _ in range(HPC)]
    h0b = qkv_blocks(0, all_tiles[0], h0_psum, act_ok=True)
    for _, b in h0b[:8]:
        b()
    # fp8 copies of x^T / w_qkv q,k-cols: needed only by the fp8 qk blocks
    # below and by drip; emitting them after the critical bf16 casts keeps
    # the first attention slot from waiting ~10us of ACT copies
    for ib in range(4):
        nc.scalar.copy(xt8_sb[:, :, ib * 512:(ib + 1) * 512],
                       xt_sb[:, :, ib * 512:(ib + 1) * 512])
    for hw in range(HPC):
        nc.scalar.copy(wqkv8_sb[:, :, hw * 512:(hw + 1) * 512],
                       wqkv_sb[:, :, hw * 3 * C:hw * 3 * C + 2 * C])
    for _, b in h0b[8:]:
        b()

    drips = []
    for h in range(1, HPC):
        base = (h - 1) * SLOTS_PER_HEAD
        blocks = qkv_blocks(h, all_tiles[h], drip_psum, act_ok=False)
        nb = len(blocks)
        for bi, (mn, cl) in enumerate(blocks):
            # avoid the DVE-saturated windows: the first 4 slots of a head
            # (qb0 + boundary normalizes) and the last 2 before the boundary
            drips.append((max(mn, base + 4 + bi * 16 // nb), cl))
    pbase = 3 * SLOTS_PER_HEAD
    for mn, cl in proj_units():
        drips.append((pbase + mn, cl))

    slots = [s for hp in range(HPC) for s in head_slots(hp)]
    work = []
    emitted = [0]

    def drip(i, limit):
        done = 0
        while (emitted[0] < len(drips) and drips[emitted[0]][0] <= i
               and done < limit):
            drips[emitted[0]][1]()
            emitted[0] += 1
            done += 1

    for i, s in enumerate(slots):
        T = all_tiles[s["hp"]]
        drip(i, 1)
        work.append((s, emit_S(s, T), T))
        if i >= 2:
            emit_PVL(*work[i - 2])
        drip(i, 1)
    emit_PVL(*work[-2])
    emit_PVL(*work[-1])
    drip(10 ** 9, 10 ** 9)


def build_program(nq=N):
    nc = bacc.Bacc(trn_type="TRN2")
    xt_d = nc.dram_tensor("xt", (C, nq), BF, kind="ExternalInput").ap()
    wqkv_d = nc.dram_tensor("wqkv", (C, 3 * HPC * C), BF, kind="ExternalInput").ap()
    wproj_d = nc.dram_tensor("wproj", (2 * HPC * 128, C), BF, kind="ExternalInput").ap()
    bias_d = nc.dram_tensor("bias", (128, 2), mybir.dt.float32, kind="ExternalInput").ap()
    mask_d = nc.dram_tensor("mask", (128, 128), BF, kind="ExternalInput").ap()
    ztri_d = nc.dram_tensor("ztri", (128, 256), BF, kind="ExternalInput").ap()
    out_d = nc.dram_tensor("out", (2 * 128, nq), mybir.dt.float32, kind="ExternalOutput").ap()
    with tile.TileContext(nc) as tc:
        import contextlib
        tc._es = contextlib.ExitStack()
        with tc._es:
            _emit(tc, nq, (xt_d, wqkv_d, wproj_d, bias_d, mask_d, ztri_d, out_d))
    nc.compile()
    return nc


def core_inputs(core, x, w_qkv, w_proj, b_proj, nq=N):
    b, hg = core // 2, core % 2
    heads = list(range(hg * HPC, hg * HPC + HPC))
    xt = np.ascontiguousarray(x[b].T).astype(BF16)
    wr = np.asarray(w_qkv, np.float32).reshape(C, 3, H, C)
    w4 = np.ascontiguousarray(
        wr[:, :, heads, :].transpose(0, 2, 1, 3)
    ).reshape(C, 3 * HPC * C).astype(BF16)
    wp = np.asarray(w_proj, np.float32).reshape(H, C, C)[heads].reshape(HPC * C, C).astype(BF16)
    bias_full = (np.asarray(b_proj, np.float32) if hg == 0
                 else np.zeros(C, np.float32))
    bias2 = np.ascontiguousarray(bias_full.reshape(2, 128).T)  # [p, f]
    tri = (np.arange(128)[:, None] <= np.arange(128)[None, :])
    mask = tri.astype(BF16)
    ztri = np.concatenate([np.zeros((128, 128), bool), tri], axis=1).astype(BF16)
    return {"xt": xt, "wqkv": w4, "wproj": wp, "bias": bias2,
            "mask": mask, "ztri": ztri}


_CACHE = {}


def kernel(x, w_qkv, w_proj, b_proj, **run_kwargs):
    x = np.asarray(x, np.float32)
    w_qkv = np.asarray(w_qkv, np.float32)
    w_proj = np.asarray(w_proj, np.float32)
    b_proj = np.asarray(b_proj, np.float32)
    if "nc" not in _CACHE:
        _CACHE["nc"] = build_program(N)
    nc = _CACHE["nc"]
    in_maps = [core_inputs(c, x, w_qkv, w_proj, b_proj) for c in range(8)]
    res = run_bass_kernel_spmd(nc, in_maps, core_ids=list(range(8)), **run_kwargs)
    out = np.zeros((B, N, C), np.float32)
    for c in range(8):
        out[c // 2] += res.results[c]["out"].T
    _CACHE["last_results"] = res
    return out


# revision 16
# speedup vs baseline: 1.2009x; 1.2009x over previous
"""Causal multi-head attention block (QKV proj + flash-style attention + out proj)
for Trainium2, sharded over 8 NeuronCores as (batch, head-group):
core c -> batch b = c//2, heads hg*4..hg*4+4 with hg = c%2.

Row-split precision hybrid per 512-wide q-block:
  qb0 (rows 0:512, concentrated softmax): bf16 everywhere (error-sensitive).
  qb1-3 (rows 512:2048): fp8(e4m3) DoubleRow matmuls for S, P@V and the
    row-sum L; exp is computed as exp(S*SCALE - SHIFT) so P <= ~60 stays
    inside TRN e4m3 range (+-240); the shift cancels in O/l exactly.
  Row sums l are computed with an all-ones [128,k] stationary so the PE
  broadcasts l to all 128 partitions (no gpsimd partition-broadcast needed).
  Out projection runs transposed (w_proj stationary, O^T moving) for
  stationary reuse; host transposes the (C, N) result back.
"""

import numpy as np
import ml_dtypes

import concourse.bass as bass
import concourse.tile as tile
from concourse import bacc, mybir
from concourse.bass_utils import run_bass_kernel_spmd

B, N, C, H = 4, 2048, 256, 8
SCALE = C ** -0.5
SHIFT = 2.0
BF16 = ml_dtypes.bfloat16
FP32 = mybir.dt.float32
BF = mybir.dt.bfloat16
F8 = mybir.dt.float8e4
HPC = 4  # heads per core
DR = mybir.MatmulPerfMode.DoubleRow


def _emit(tc, nq, aps):
    nc = tc.nc
    nt = nq // 128

    xt_d, wqkv_d, wproj_d, bias_d, mask_d, ztri_d, out_d = aps
    xt_r = xt_d.rearrange("(c p) n -> p c n", p=128)
    wqkv_r = wqkv_d.rearrange("(c p) m -> p c m", p=128)
    wproj_r = wproj_d.rearrange("(t p) f -> p t f", p=128)
    out_r = out_d.rearrange("(t p) n -> p t n", p=128)

    singles = tc._es.enter_context(tc.tile_pool(name="singles", bufs=1))
    pool_head = tc._es.enter_context(tc.tile_pool(name="headp", bufs=2))
    pool_p16 = tc._es.enter_context(tc.tile_pool(name="p16p", bufs=3))
    pool_p8 = tc._es.enter_context(tc.tile_pool(name="p8p", bufs=3))
    pool_rb = tc._es.enter_context(tc.tile_pool(name="rbp", bufs=2))
    pool_osb = tc._es.enter_context(tc.tile_pool(name="osbp", bufs=4))
    # PSUM: sp 2x2 banks + ot0/ot1 2 + l 1 + qp 1 = 8 banks
    pool_sp = tc._es.enter_context(tc.tile_pool(name="psumsp", bufs=2, space="PSUM"))
    pool_ot = tc._es.enter_context(tc.tile_pool(name="psumot", bufs=1, space="PSUM"))
    pool_l = tc._es.enter_context(tc.tile_pool(name="psuml", bufs=1, space="PSUM"))
    pool_qp = tc._es.enter_context(tc.tile_pool(name="psumqp", bufs=1, space="PSUM"))

    # --- SBUF constants / inputs ---
    xt_sb = singles.tile([128, 2, nq], BF)
    wqkv_sb = singles.tile([128, 2, 3 * HPC * C], BF)
    xt8_sb = singles.tile([128, 2, nq], F8)
    wqkv8_sb = singles.tile([128, 2, 2 * HPC * C], F8)  # q,k cols only
    wproj_sb = singles.tile([128, 2 * HPC, C], BF)
    bias_sb = singles.tile([128, 2], FP32)
    mask_sb = singles.tile([128, 128], BF)
    ztri_sb = singles.tile([128, 256], BF)
    tri8_sb = singles.tile([128, 128], F8)
    ztri8_sb = singles.tile([128, 256], F8)
    ones16_sb = singles.tile([128, 128], BF)
    ones8_sb = singles.tile([128, 2, 128], F8)
    shiftb_sb = singles.tile([128, 1], FP32)
    ot_sb = singles.tile([128, 2 * HPC, nq], BF)

    # input DMAs split across both HW-DGE rings; the small mask DMA goes
    # first so the PE warm-up (which reads it) can start ~3us in
    nc.sync.dma_start(mask_sb[:], mask_d[:])
    for ib in range(4):
        nc.sync.dma_start(xt_sb[:, :, ib * 512:(ib + 1) * 512],
                          xt_r[:, :, ib * 512:(ib + 1) * 512])
    for hw in range(HPC):
        c0 = hw * 3 * C
        nc.scalar.dma_start(wqkv_sb[:, :, c0:c0 + 3 * C], wqkv_r[:, :, c0:c0 + 3 * C])
    nc.sync.dma_start(ztri_sb[:], ztri_d[:])
    nc.scalar.dma_start(wproj_sb[:], wproj_r)
    nc.scalar.dma_start(bias_sb[:], bias_d[:])
    # fp8 copies of x^T / w_qkv q,k-cols, split in halves and alternated
    # across ACT/DVE so neither engine serializes ~10us of copies while the
    # PE waits on head-0's fp8 qk blocks
    cp_i = 0
    for ib in range(4):
        for h2 in range(2):
            lo = ib * 512 + h2 * 256
            if cp_i % 2 == 0:
                nc.scalar.copy(xt8_sb[:, :, lo:lo + 256], xt_sb[:, :, lo:lo + 256])
            else:
                nc.vector.tensor_copy(xt8_sb[:, :, lo:lo + 256],
                                      xt_sb[:, :, lo:lo + 256])
            cp_i += 1
    for hw in range(HPC):
        for h2 in range(2):
            lo8, lo16 = hw * 512 + h2 * 256, hw * 3 * C + h2 * 256
            if cp_i % 2 == 0:
                nc.scalar.copy(wqkv8_sb[:, :, lo8:lo8 + 256],
                               wqkv_sb[:, :, lo16:lo16 + 256])
            else:
                nc.vector.tensor_copy(wqkv8_sb[:, :, lo8:lo8 + 256],
                                      wqkv_sb[:, :, lo16:lo16 + 256])
            cp_i += 1
    nc.vector.tensor_copy(tri8_sb[:], mask_sb[:])
    nc.vector.tensor_copy(ztri8_sb[:], ztri_sb[:])
    nc.gpsimd.memset(ones16_sb[:], 1.0)
    nc.gpsimd.memset(ones8_sb[:], 1.0)
    nc.gpsimd.memset(shiftb_sb[:], -SHIFT)

    # warm up the PE HAM clock gate while the big input DMAs land; gated
    # only on the tiny mask DMA so the matmuls start ~3us into the kernel
    warm_ps = pool_qp.tile([128, 512], FP32, tag="qp", name="warm_ps")
    for wi in range(12):
        nc.tensor.matmul(warm_ps[:, :128], mask_sb[:], mask_sb[:],
                         start=(wi == 0), stop=(wi == 11))

    def alloc_head_tiles():
        qt16 = pool_head.tile([128, 2, 512], BF, tag="qt16", name="qt16")
        kt16 = pool_head.tile([128, 2, 512], BF, tag="kt16", name="kt16")
        qt8 = pool_head.tile([128, 2, nq], F8, tag="qt8", name="qt8")
        kt8 = pool_head.tile([128, 2, nq], F8, tag="kt8", name="kt8")
        v16 = pool_head.tile([128, 4, C], BF, tag="v16", name="v16")
        v8 = pool_head.tile([128, nt, C], F8, tag="v8", name="v8")
        return qt16, kt16, qt8, kt8, v16, v8

    par = {"i": 0}

    def qkv_blocks(hp, tiles, psum_alloc, act_ok):
        """Per-(128x512)-block closures for head hp's QKV projection.
        psum_alloc() -> a [128, 512] fp32 PSUM tile.  act_ok: may use the
        Scalar engine for casts (only before attention starts; during
        attention ACT must stay exp-only or drip casts stall the PE)."""
        qt16, kt16, qt8, kt8, v16, v8 = tiles
        blocks = []

        def cast(dst, src):
            par["i"] += 1
            if act_ok and par["i"] % 2 == 0:
                nc.scalar.copy(dst, src)
            else:
                nc.vector.tensor_copy(dst, src)

        def qk_block(j, ct, ib):
            def go():
                ps = psum_alloc()
                tgt8 = qt8 if j == 0 else kt8
                if ib == 0:
                    col0 = (hp * 3 + j) * C + ct * 128
                    for ci in range(2):
                        nc.tensor.matmul(
                            ps[:], wqkv_sb[:, ci, col0:col0 + 128],
                            xt_sb[:, ci, ib * 512:(ib + 1) * 512],
                            start=(ci == 0), stop=(ci == 1),
                        )
                    tgt16 = qt16 if j == 0 else kt16
                    cast(tgt16[:, ct, :], ps[:])
                    if j == 1:
                        cast(tgt8[:, ct, 0:512], ps[:])
                else:
                    col8 = (hp * 2 + j) * C + ct * 128
                    nc.tensor.matmul(
                        ps[:], wqkv8_sb[:, :, col8:col8 + 128],
                        xt8_sb[:, :, ib * 512:(ib + 1) * 512],
                        start=True, stop=True, perf_mode=DR,
                    )
                    cast(tgt8[:, ct, ib * 512:(ib + 1) * 512], ps[:])
            return go

        def v_block(it):
            def go():
                ps = psum_alloc()
                vcol = (hp * 3 + 2) * C
                for ci in range(2):
                    nc.tensor.matmul(
                        ps[:, :C], xt_sb[:, ci, it * 128:(it + 1) * 128],
                        wqkv_sb[:, ci, vcol:vcol + C],
                        start=(ci == 0), stop=(ci == 1),
                    )
                cast(v8[:, it, :], ps[:, :C])
                if it < 4:
                    cast(v16[:, it, :], ps[:, :C])
            return go

        # order: qb0-critical first (q/k ib0, v it0-3), then fp8 operands
        for j in (0, 1):
            for ct in range(2):
                blocks.append((0, qk_block(j, ct, 0)))
        for it in range(4):
            blocks.append((0, v_block(it)))
        for j in (1, 0):
            for ct in range(2):
                for ib in range(1, 4):
                    blocks.append((0, qk_block(j, ct, ib)))
        for it in range(4, nt):
            blocks.append((0, v_block(it)))
        return blocks

    otl_rot = {"i": 0}
    otl_tags = [(pool_ot, "ot0"), (pool_ot, "ot1"), (pool_l, "l")]
    SLOTS_PER_HEAD = 22

    def head_slots(hp):
        slots = []
        for kt in range(4):  # qb0, bf16, one kt per slot
            q_off = kt * 128
            slots.append(dict(hp=hp, kind=16, qb=0, kt=kt, q_off=q_off,
                              nqf=512 - q_off, first=(kt == 0), last=(kt == 3),
                              diag=True))
        for qb in range(1, 4):  # fp8 pairs
            npair = 2 * qb + 2
            for j in range(npair):
                q_off = 256 if j == npair - 1 else 0
                slots.append(dict(hp=hp, kind=8, qb=qb, j=j, q_off=q_off,
                                  nqf=512 - q_off, first=(j == 0),
                                  last=(j == npair - 1), diag=(j >= 2 * qb)))
        return slots

    state = {}

    def otl_tiles():
        """Allocate (ot0, ot1, lp) with per-qb tag rotation so each new
        allocation lands on the earliest-freed PSUM bank."""
        r = otl_rot["i"]
        otl_rot["i"] += 1
        tags = [otl_tags[(r + k) % 3] for k in range(3)]
        # emission order per first slot: ot1 MM, ot0 MM, L MM
        ot1 = tags[0][0].tile([128, 512], FP32, tag=tags[0][1], name="ot1")
        ot0 = tags[1][0].tile([128, 512], FP32, tag=tags[1][1], name="ot0")
        lp = tags[2][0].tile([128, 512], FP32, tag=tags[2][1], name="lp")
        return ot0, ot1, lp

    def emit_S(s, T):
        qt16, kt16, qt8, kt8, v16, v8 = T
        sp = pool_sp.tile([128, 2, 512], FP32, tag="sp", name="sp")
        nqf = s["nqf"]
        if s["kind"] == 16:
            q0 = s["q_off"]
            for ci in range(2):
                nc.tensor.matmul(
                    sp[:, 0, :nqf], kt16[:, ci, s["kt"] * 128:(s["kt"] + 1) * 128],
                    qt16[:, ci, q0:q0 + nqf], start=(ci == 0), stop=(ci == 1),
                )
            p = pool_p16.tile([128, 512], BF, tag="p16", name="p16")
            nc.scalar.activation(p[:, :nqf], sp[:, 0, :nqf],
                                 mybir.ActivationFunctionType.Exp, scale=SCALE)
            nc.vector.tensor_tensor(p[:, :128], p[:, :128], mask_sb[:],
                                    mybir.AluOpType.mult)
        else:
            q0 = s["qb"] * 512 + s["q_off"]
            p = pool_p8.tile([128, 2, 512], F8, tag="p8", name="p8")
            for i in range(2):
                kt = 2 * s["j"] + i
                nc.tensor.matmul(
                    sp[:, i, :nqf], kt8[:, :, kt * 128:(kt + 1) * 128],
                    qt8[:, :, q0:q0 + nqf], start=True, stop=True, perf_mode=DR,
                )
            nc.scalar.activation(p[:, :, :nqf], sp[:, :, :nqf],
                                 mybir.ActivationFunctionType.Exp,
                                 scale=SCALE, bias=shiftb_sb[:])
            if s["diag"]:
                nc.vector.tensor_tensor(p[:, 0, 0:128], p[:, 0, 0:128],
                                        tri8_sb[:], mybir.AluOpType.mult)
                nc.vector.tensor_tensor(p[:, 1, 0:256], p[:, 1, 0:256],
                                        ztri8_sb[:], mybir.AluOpType.mult)
        return p

    def emit_PVL(s, p, T):
        qt16, kt16, qt8, kt8, v16, v8 = T
        hp, qb, q_off, nqf = s["hp"], s["qb"], s["q_off"], s["nqf"]
        first, last = s["first"], s["last"]
        if first:
            state[(hp, qb)] = (*otl_tiles(), {})
        ot0, ot1, lp, aux = state[(hp, qb)]
        # groups close ("stop") once cols [0:256) are final so the half
        # normalize may read them; later matmuls keep accumulating the
        # [256:512) range (stop is sim bookkeeping only, no HW effect)
        if s["kind"] == 16:
            kt = s["kt"]
            stop_f = (kt == 1) or last
            skip = kt >= 2
            nc.tensor.matmul(ot1[:, q_off:], v16[:, kt, 0:128], p[:, :nqf],
                             start=first, stop=stop_f, skip_group_check=skip)
            nc.tensor.matmul(ot0[:, q_off:], v16[:, kt, 128:256], p[:, :nqf],
                             start=first, stop=stop_f, skip_group_check=skip)
            nc.tensor.matmul(lp[:, q_off:], ones16_sb[:], p[:, :nqf],
                             start=first, stop=stop_f, skip_group_check=skip)
        else:
            j2 = 2 * s["j"]
            stop_f = s["diag"]
            skip = last
            nc.tensor.matmul(ot1[:, q_off:], v8[:, j2:j2 + 2, 0:128],
                             p[:, :, :nqf], start=first, stop=stop_f,
                             skip_group_check=skip, perf_mode=DR)
            nc.tensor.matmul(ot0[:, q_off:], v8[:, j2:j2 + 2, 128:256],
                             p[:, :, :nqf], start=first, stop=stop_f,
                             skip_group_check=skip, perf_mode=DR)
            nc.tensor.matmul(lp[:, q_off:], ones8_sb[:],
                             p[:, :, :nqf], start=first, stop=stop_f,
                             skip_group_check=skip, perf_mode=DR)
        # normalize in halves: cols [0:256) are final one slot before the
        # last (the final pair/step only writes cols >= 256), which spreads
        # the DVE work and frees the ot/l banks earlier
        def half_norm(lo, hi):
            if aux.get("rb") is None:
                aux["rb"] = pool_rb.tile([128, 512], FP32, tag="rb", name="rb")
            rb = aux["rb"]
            nc.vector.reciprocal_approx_fast(rb[:, lo:hi], lp[:, lo:hi])
            for ct, otp in ((0, ot1), (1, ot0)):
                nc.vector.tensor_tensor(
                    ot_sb[:, hp * 2 + ct, qb * 512 + lo:qb * 512 + hi],
                    otp[:, lo:hi], rb[:, lo:hi], mybir.AluOpType.mult,
                )

        if (s["kind"] == 16 and s["kt"] == 1) or (s["kind"] == 8 and s["diag"]
                                                  and not last):
            half_norm(0, 256)
        if last:
            half_norm(256, 512)

    # --- head 0 QKV with a deep temporary psum ring (attention not started) ---
    h0_ring = {"i": 0}
    h0_tags = [(pool_qp, "qp"), (pool_ot, "ot0"), (pool_ot, "ot1"), (pool_l, "l")]

    def h0_psum():
        pool, tag = h0_tags[h0_ring["i"] % 4]
        h0_ring["i"] += 1
        return pool.tile([128, 512], FP32, tag=tag, name="h0qkv")

    def drip_psum():
        return pool_qp.tile([128, 512], FP32, tag="qp", name="qkvps")

    def proj_units():
        """Output projection out^T[f, n] = sum_t W[t]^T O^T[t] + b, one unit
        per (f-chunk, n-chunk), dripped into head 3's attention as soon as
        head 3's q-block n-chunk has been normalized."""
        units = []

        def unit(f, nch):
            def go():
                ps = pool_qp.tile([128, 512], FP32, tag="qp", name="ps_prj")
                for t in range(2 * HPC):
                    nc.tensor.matmul(
                        ps[:], wproj_sb[:, t, f * 128:(f + 1) * 128],
                        ot_sb[:, t, nch * 512:(nch + 1) * 512],
                        start=(t == 0), stop=(t == 2 * HPC - 1),
                    )
                osb = pool_osb.tile([128, 512], FP32, tag="osb", name="osb")
                nc.vector.tensor_scalar_add(osb[:], ps[:], bias_sb[:, f:f + 1])
                nc.sync.dma_start(out_r[:, f, nch * 512:(nch + 1) * 512], osb[:])
            return go

        mins = {0: 6, 1: 10, 2: 16, 3: 22}
        for nch in range(4):
            for f in range(2):
                units.append((mins[nch], unit(f, nch)))
        return units

    # --- global slot stream: the lag-2 S->PVL pipeline runs across all four
    # heads so there is no per-head drain; head h+1's QKV (and the output
    # projection during head 3) drip into the stream paced by min-slot ---
    all_tiles = [alloc_head_tiles() for _ in range(HPC)]
    for _, b in qkv_blocks(0, all_tiles[0], h0_psum, act_ok=True):
        b()

    drips = []
    for h in range(1, HPC):
        base = (h - 1) * SLOTS_PER_HEAD
        blocks = qkv_blocks(h, all_tiles[h], drip_psum, act_ok=False)
        nb = len(blocks)
        for bi, (mn, cl) in enumerate(blocks):
            # avoid the DVE-saturated windows: the first 4 slots of a head
            # (qb0 + boundary normalizes) and the last 2 before the boundary
            drips.append((max(mn, base + 4 + bi * 16 // nb), cl))
    pbase = 3 * SLOTS_PER_HEAD
    for mn, cl in proj_units():
        drips.append((pbase + mn, cl))

    slots = [s for hp in range(HPC) for s in head_slots(hp)]
    work = []
    emitted = [0]

    def drip(i, limit):
        done = 0
        while (emitted[0] < len(drips) and drips[emitted[0]][0] <= i
               and done < limit):
            drips[emitted[0]][1]()
            emitted[0] += 1
            done += 1

    for i, s in enumerate(slots):
        T = all_tiles[s["hp"]]
        drip(i, 1)
        work.append((s, emit_S(s, T), T))
        if i >= 2:
            emit_PVL(*work[i - 2])
        drip(i, 1)
    emit_PVL(*work[-2])
    emit_PVL(*work[-1])
    drip(10 ** 9, 10 ** 9)


def build_program(nq=N):
    nc = bacc.Bacc(trn_type="TRN2")
    xt_d = nc.dram_tensor("xt", (C, nq), BF, kind="ExternalInput").ap()
    wqkv_d = nc.dram_tensor("wqkv", (C, 3 * HPC * C), BF, kind="ExternalInput").ap()
    wproj_d = nc.dram_tensor("wproj", (2 * HPC * 128, C), BF, kind="ExternalInput").ap()
    bias_d = nc.dram_tensor("bias", (128, 2), mybir.dt.float32, kind="ExternalInput").ap()
    mask_d = nc.dram_tensor("mask", (128, 128), BF, kind="ExternalInput").ap()
    ztri_d = nc.dram_tensor("ztri", (128, 256), BF, kind="ExternalInput").ap()
    out_d = nc.dram_tensor("out", (2 * 128, nq), mybir.dt.float32, kind="ExternalOutput").ap()
    with tile.TileContext(nc) as tc:
        import contextlib
        tc._es = contextlib.ExitStack()
        with tc._es:
            _emit(tc, nq, (xt_d, wqkv_d, wproj_d, bias_d, mask_d, ztri_d, out_d))
    nc.compile()
    return nc


def core_inputs(core, x, w_qkv, w_proj, b_proj, nq=N):
    b, hg = core // 2, core % 2
    heads = list(range(hg * HPC, hg * HPC + HPC))
    xt = np.ascontiguousarray(x[b].T).astype(BF16)
    wr = np.asarray(w_qkv, np.float32).reshape(C, 3, H, C)
    w4 = np.ascontiguousarray(
        wr[:, :, heads, :].transpose(0, 2, 1, 3)
    ).reshape(C, 3 * HPC * C).astype(BF16)
    wp = np.asarray(w_proj, np.float32).reshape(H, C, C)[heads].reshape(HPC * C, C).astype(BF16)
    bias_full = (np.asarray(b_proj, np.float32) if hg == 0
                 else np.zeros(C, np.float32))
    bias2 = np.ascontiguousarray(bias_full.reshape(2, 128).T)  # [p, f]
    tri = (np.arange(128)[:, None] <= np.arange(128)[None, :])
    mask = tri.astype(BF16)
    ztri = np.concatenate([np.zeros((128, 128), bool), tri], axis=1).astype(BF16)
    return {"xt": xt, "wqkv": w4, "wproj": wp, "bias": bias2,
            "mask": mask, "ztri": ztri}


_CACHE = {}


def kernel(x, w_qkv, w_proj, b_proj, **run_kwargs):
    x = np.asarray(x, np.float32)
    w_qkv = np.asarray(w_qkv, np.float32)
    w_proj = np.asarray(w_proj, np.float32)
    b_proj = np.asarray(b_proj, np.float32)
    if "nc" not in _CACHE:
        _CACHE["nc"] = build_program(N)
    nc = _CACHE["nc"]
    in_maps = [core_inputs(c, x, w_qkv, w_proj, b_proj) for c in range(8)]
    res = run_bass_kernel_spmd(nc, in_maps, core_ids=list(range(8)), **run_kwargs)
    out = np.zeros((B, N, C), np.float32)
    for c in range(8):
        out[c // 2] += res.results[c]["out"].T
    _CACHE["last_results"] = res
    return out
